# revision 20
# baseline (speedup 1.0000x reference)
"""GATv2 localization model on 8 Trainium2 NeuronCores (Bass/Tile), v3.

Strategy (dst-sharded message passing, bf16 edge pipeline):
  - Nodes sharded across 8 cores by dst (6250 each); per core, nodes are
    degree-sorted into 49 blocks of 128 and incoming edges are packed into
    per-block slot columns (slot q of block b holds edge q of each node).
  - The attention input y[e] = xl[src] + xr[dst] + We*ea is assembled on
    the host in slot order and streamed sequentially (one direct DMA per
    block) — on TRN2, software-DGE indirect gathers cost ~6ns/row on the
    gpsimd sequencer, so a prearranged affine stream is the only way to
    run this at memory bandwidth.
  - Per pass (8 slots): scalar applies leaky-relu, vector applies att and
    reduces to logits. Softmax (exp/mask/denominator) runs once per
    block; alpha is pre-normalized so no divide is needed downstream.
  - Aggregation uses sum_j alpha_j*xl_j = sum_j alpha_j*y_j - xr -
    (sum_j alpha_j ea_j)*We: gpsimd forms alpha*y, vector pair-sums
    slots, and the tensor engine accumulates into PSUM via identity-lhsT
    matmuls; the correction terms are per-block vector ops.
  - Layer-1 tables are host-computed; layer-2 tables are built on-device
    from h1 (bf16, true ELU to keep bf16 precision) and exchanged through
    the host between the two launches. MLP head on-device per block.
"""

import os
import numpy as np
import ml_dtypes

import concourse.bacc as bacc
import concourse.tile as tile
import concourse.mybir as mybir
from concourse import bass
from concourse.bass_utils import run_bass_kernel_spmd
from concourse.masks import make_identity

F32 = mybir.dt.float32
BF16 = mybir.dt.bfloat16
BF = ml_dtypes.bfloat16

N = 50000
E = 800000
IN = 16
H1 = 4
C1 = 32
HC = 128
OUT = 2
NCORES = 8
NSHARD = N // NCORES          # 6250
NBLK = (NSHARD + 127) // 128  # 49
NPAD = NBLK * 128             # 6272
K = 8                         # slots per pass

_EXEC_NS = []                 # per-launch HW exec time when GAT_TRACE=1


def _maybe_install_trace_hook():
    if os.environ.get("GAT_TRACE", "0") != "1":
        return False
    import contextlib, ctypes, sys, types
    if "antenv.axon_hooks" not in sys.modules:
        def _mk(so_path):
            lib = ctypes.CDLL(so_path)
            if not hasattr(lib, "axon_start_nrt_profile"):
                return None
            lib.axon_start_nrt_profile.argtypes = [ctypes.POINTER(ctypes.c_int64), ctypes.c_size_t]
            lib.axon_start_nrt_profile.restype = ctypes.c_int64
            lib.axon_stop_nrt_profile.argtypes = [ctypes.c_char_p]
            lib.axon_stop_nrt_profile.restype = ctypes.c_int64

            @contextlib.contextmanager
            def _hook(output_dir, device_ids):
                import jax
                jax.devices()
                if device_ids:
                    ids = (ctypes.c_int64 * len(device_ids))(*device_ids)
                    rc = lib.axon_start_nrt_profile(ids, len(device_ids))
                else:
                    rc = lib.axon_start_nrt_profile(None, 0)
                if rc != 0:
                    raise RuntimeError(f"axon_start_nrt_profile rc={rc}")
                try:
                    yield
                finally:
                    n = lib.axon_stop_nrt_profile(str(output_dir).encode())
                    if n < 0:
                        raise RuntimeError(f"axon_stop_nrt_profile rc={n}")
            return _hook

        hook = _mk("/opt/axon/libaxon_pjrt.so")
        mod = types.ModuleType("antenv.axon_hooks")
        mod.get_axon_ntff_profile_hook = lambda: hook
        mod.set_axon_ntff_profile_hook = lambda h: None
        sys.modules["antenv.axon_hooks"] = mod
        import concourse.bass_utils as bu
        bu.upload_artifacts = lambda tmpdir: tmpdir
    return True


def _run(nc, in_maps):
    trace = _maybe_install_trace_hook()
    if trace:
        import tempfile
        res = run_bass_kernel_spmd(nc, in_maps, core_ids=list(range(NCORES)),
                                   trace=True, tmpdir=tempfile.mkdtemp())
        _EXEC_NS.append(res.exec_time_ns)
    else:
        res = run_bass_kernel_spmd(nc, in_maps, core_ids=list(range(NCORES)))
    return res.results


# ---------------------------------------------------------------- schedule

def _build_schedule(edge_index, edge_attr):
    """Degree-sorted blocks, slot layout, per-core slot-ordered edge refs."""
    src = edge_index[0].astype(np.int64)
    dst = edge_index[1].astype(np.int64)
    ea = edge_attr[:, 0].astype(np.float32)

    deg = np.bincount(dst, minlength=N)
    cores = []
    for k in range(NCORES):
        lo, hi = k * NSHARD, (k + 1) * NSHARD
        nodes = np.arange(lo, hi)
        order = np.argsort(-deg[lo:hi], kind="stable")
        perm = nodes[order]                       # block row -> global node id
        perm_pad = np.concatenate([perm, np.full(NPAD - NSHARD, -1, np.int64)])
        cores.append({"perm_pad": perm_pad})

    # shared slot counts per block (max over cores)
    SLOTS = np.zeros(NBLK, np.int64)
    for k in range(NCORES):
        perm_pad = cores[k]["perm_pad"]
        d = np.where(perm_pad >= 0, deg[np.clip(perm_pad, 0, N - 1)], 0)
        SLOTS = np.maximum(SLOTS, d.reshape(NBLK, 128).max(axis=1))
    SLOTS = np.maximum(SLOTS, 1)
    PB = (SLOTS + K - 1) // K
    scol0 = np.zeros(NBLK + 1, np.int64)
    scol0[1:] = np.cumsum(SLOTS)
    TOTS = int(SLOTS.sum())

    # edge lists grouped by dst
    e_order = np.argsort(dst, kind="stable")
    src_s, ea_s = src[e_order], ea[e_order]
    starts = np.searchsorted(dst[e_order], np.arange(N + 1))

    # per-core slot-ordered edge references: for slot column (b, q), row r:
    # flat position p = (scol0[b]+q)*128 + r -> source node id (or -1 pad)
    for k in range(NCORES):
        perm_pad = cores[k]["perm_pad"]
        srcmap = np.full(TOTS * 128, -1, np.int64)   # [(scol, r)]
        eamap = np.zeros((128, TOTS), np.float32)    # [r, scol]
        mkmap = np.zeros((128, TOTS), np.float32)
        for b in range(NBLK):
            rows = perm_pad[b * 128:(b + 1) * 128]
            for r in range(128):
                n = rows[r]
                if n < 0:
                    continue
                s0e, s1e = starts[n], starts[n + 1]
                d = s1e - s0e
                if d == 0:
                    continue
                cols = scol0[b] + np.arange(d)
                srcmap[cols * 128 + r] = src_s[s0e:s1e]
                eamap[r, cols] = ea_s[s0e:s1e]
                mkmap[r, cols] = 1.0
        cores[k]["srcmap"] = srcmap
        cores[k]["eamap"] = eamap
        cores[k]["mkblob"] = mkmap
        cores[k]["eablob"] = eamap.astype(BF)

    shared = {"SLOTS": SLOTS, "PB": PB, "scol0": scol0, "TOTS": TOTS}
    return cores, shared


def _ytab_blob(core, xl, xr, we, att_flat, shared):
    """[128, TOTS*HC] bf16: y = xl[src]+xr[dst]+We*ea in slot order.

    Pad slots are filled with -t*sign(att) per channel, which makes the
    device-computed logit sum(att*lrelu(y)) <= -40 for every head, so
    exp() underflows to ~0 and no mask multiply is needed."""
    TOTS = shared["TOTS"]
    srcmap = core["srcmap"]                  # [(scol)*128 + r]
    perm_pad = core["perm_pad"]
    safe_src = np.clip(srcmap, 0, N - 1)
    yt = xl[safe_src].astype(np.float32)     # [(scol, r), HC]
    yt = yt.reshape(TOTS, 128, HC)
    # xr[dst]: dst of (b, q, r) is block row r of block b
    scol0 = shared["scol0"]
    safe = np.clip(perm_pad, 0, N - 1)
    v = xr[safe].astype(np.float32)
    v[perm_pad < 0] = 0.0
    xrv = v.reshape(NBLK, 128, HC)
    for b in range(NBLK):
        yt[scol0[b]:scol0[b + 1]] += xrv[b][None, :, :]
    yt += core["eamap"].T[:, :, None] * we[None, None, :]
    # pad-slot poison logits: coefficient per head of -t is
    # sum_c(0.2*max(att,0) - min(att,0)); choose t so every head's pad
    # logit is <= -40.
    a = att_flat.reshape(-1).astype(np.float32)
    heads = H1 if shared["_layer"] == 1 else 1
    ah = a.reshape(heads, -1)
    coef = (0.2 * np.maximum(ah, 0) - np.minimum(ah, 0)).sum(axis=1)
    t = min(40.0 / max(coef.min(), 1e-3), 3e4)
    padvec = (-t * np.sign(a)).astype(np.float32)
    yt[srcmap.reshape(TOTS, 128) < 0] = padvec
    return np.ascontiguousarray(yt.transpose(1, 0, 2).reshape(128, TOTS * HC)).astype(BF)


# ---------------------------------------------------------------- launches

def _build_launch(layer, shared):
    nc = bacc.Bacc("TRN2", target_bir_lowering=False, debug=False,
                   num_devices=NCORES)
    H = H1 if layer == 1 else 1
    C = HC // H
    SLOTS, PB, scol0 = shared["SLOTS"], shared["PB"], shared["scol0"]
    TOTS = shared["TOTS"]
    SMAX = int(SLOTS.max())

    t_ytab = nc.dram_tensor("t_ytab", [128, TOTS * HC], BF16, kind="ExternalInput")
    t_ea = nc.dram_tensor("t_ea", [128, TOTS], BF16, kind="ExternalInput")
    t_xrb = nc.dram_tensor("t_xrb", [128, NBLK * HC], F32, kind="ExternalInput")
    t_webf = nc.dram_tensor("t_webf", [128, HC], F32, kind="ExternalInput")
    t_attb = nc.dram_tensor("t_attb", [128, HC], BF16, kind="ExternalInput")
    if layer == 1:
        t_wl2 = nc.dram_tensor("t_wl2", [HC, HC], BF16, kind="ExternalInput")
        t_wr2 = nc.dram_tensor("t_wr2", [HC, HC], BF16, kind="ExternalInput")
        t_bl2row = nc.dram_tensor("t_bl2row", [128, HC], F32, kind="ExternalInput")
        t_br2row = nc.dram_tensor("t_br2row", [128, HC], F32, kind="ExternalInput")
        o_xl2 = nc.dram_tensor("o_xl2", [NPAD, HC], BF16, kind="ExternalOutput")
        o_xr2 = nc.dram_tensor("o_xr2", [NBLK, 128, HC], BF16, kind="ExternalOutput")
    else:
        t_w1 = nc.dram_tensor("t_w1", [HC, 32], BF16, kind="ExternalInput")
        t_w2 = nc.dram_tensor("t_w2", [32, 32], BF16, kind="ExternalInput")
        t_w3 = nc.dram_tensor("t_w3", [32, OUT], BF16, kind="ExternalInput")
        t_c1 = nc.dram_tensor("t_c1", [32, 1], F32, kind="ExternalInput")
        t_c2 = nc.dram_tensor("t_c2", [32, 1], F32, kind="ExternalInput")
        t_c3 = nc.dram_tensor("t_c3", [OUT, 1], F32, kind="ExternalInput")
        o_out = nc.dram_tensor("o_out", [NBLK, OUT, 128], F32, kind="ExternalOutput")

    with tile.TileContext(nc) as tc:
        with tc.tile_pool(name="const", bufs=1) as cpool, \
             tc.tile_pool(name="blk", bufs=2) as bpool, \
             tc.tile_pool(name="pas", bufs=3) as ppool, \
             tc.tile_pool(name="psum", bufs=1, space="PSUM") as psum:
            webf = cpool.tile([128, HC], F32)
            nc.sync.dma_start(out=webf[:], in_=t_webf.ap())
            attb = cpool.tile([128, HC], BF16)
            nc.sync.dma_start(out=attb[:], in_=t_attb.ap())
            eab = cpool.tile([128, TOTS], BF16)
            nc.sync.dma_start(out=eab[:], in_=t_ea.ap())
            xrbb = cpool.tile([128, NBLK * HC], F32)
            nc.sync.dma_start(out=xrbb[:], in_=t_xrb.ap())
            identbf = cpool.tile([128, 128], BF16)
            make_identity(nc, identbf[:])
            if layer == 1:
                wl2 = cpool.tile([HC, HC], BF16)
                nc.sync.dma_start(out=wl2[:], in_=t_wl2.ap())
                wr2 = cpool.tile([HC, HC], BF16)
                nc.sync.dma_start(out=wr2[:], in_=t_wr2.ap())
                bl2row = cpool.tile([128, HC], F32)
                nc.sync.dma_start(out=bl2row[:], in_=t_bl2row.ap())
                br2row = cpool.tile([128, HC], F32)
                nc.sync.dma_start(out=br2row[:], in_=t_br2row.ap())
            else:
                w1 = cpool.tile([HC, 32], BF16)
                nc.sync.dma_start(out=w1[:], in_=t_w1.ap())
                w2 = cpool.tile([32, 32], BF16)
                nc.sync.dma_start(out=w2[:], in_=t_w2.ap())
                w3 = cpool.tile([32, OUT], BF16)
                nc.sync.dma_start(out=w3[:], in_=t_w3.ap())
                c1 = cpool.tile([32, 1], F32)
                nc.sync.dma_start(out=c1[:], in_=t_c1.ap())
                c2 = cpool.tile([32, 1], F32)
                nc.sync.dma_start(out=c2[:], in_=t_c2.ap())
                c3 = cpool.tile([OUT, 1], F32)
                nc.sync.dma_start(out=c3[:], in_=t_c3.ap())

            for b in range(NBLK):
                S = int(SLOTS[b])
                s0 = int(scol0[b])
                yblk = bpool.tile([128, SMAX, HC], BF16, tag="yblk")
                nc.sync.dma_start(
                    out=yblk[:, :S, :],
                    in_=t_ytab.ap()[:, s0 * HC:(s0 + S) * HC]
                        .rearrange("p (s c) -> p s c", c=HC))
                wall = bpool.tile([128, SMAX * H], F32, tag="wall")

                for pl in range(int(PB[b])):
                    j0 = pl * K
                    sp = min(K, S - j0)
                    m = ppool.tile([128, K, HC], BF16, tag="m")
                    nc.scalar.activation(out=m[:, :sp, :], in_=yblk[:, j0:j0 + sp, :],
                                         func=mybir.ActivationFunctionType.Prelu,
                                         alpha=0.2)
                    mm = ppool.tile([128, K, HC], BF16, tag="mm")
                    # balance the att multiply across vector / gpsimd
                    eng = nc.vector if pl % 2 == 0 else nc.gpsimd
                    eng.tensor_mul(
                        out=mm[:, :sp, :], in0=m[:, :sp, :],
                        in1=attb[:].unsqueeze(1).broadcast_to([128, sp, HC]))
                    nc.vector.tensor_reduce(
                        out=wall[:, j0 * H:(j0 + sp) * H],
                        in_=mm[:, :sp, :].rearrange("p j (h c) -> p j h c", h=H),
                        axis=mybir.AxisListType.X, op=mybir.AluOpType.add)

                # ---- block-level softmax weights (pre-normalized alpha)
                # (pad slots carry logit ~ -46 from the host ytab, so no mask
                #  multiply and no epsilon are needed)
                nc.scalar.activation(out=wall[:, :S * H], in_=wall[:, :S * H],
                                     func=mybir.ActivationFunctionType.Exp)
                dacc = bpool.tile([128, H], F32, tag="dacc")
                nc.vector.tensor_reduce(
                    out=dacc[:],
                    in_=wall[:, :S * H].rearrange("p (j h) -> p h j", j=S),
                    axis=mybir.AxisListType.X, op=mybir.AluOpType.add)
                nc.vector.reciprocal(out=dacc[:], in_=dacc[:])
                alpha = bpool.tile([128, SMAX * H], BF16, tag="alpha")
                for h in range(H):
                    nc.scalar.activation(
                        out=alpha[:, :S * H].rearrange("p (j h) -> p h j", j=S)[:, h, :],
                        in_=wall[:, :S * H].rearrange("p (j h) -> p h j", j=S)[:, h, :],
                        func=mybir.ActivationFunctionType.Copy,
                        scale=dacc[:, h:h + 1])

                # ---- phase 2: Sum_j alpha_j*y_j via gpsimd mul + pair-sum + PE
                hps = psum.tile([128, HC], F32, tag="hps")
                nmm = 0
                mm_total = sum((min(K, S - pl * K) + 1) // 2
                               for pl in range(int(PB[b])))
                for pl in range(int(PB[b])):
                    j0 = pl * K
                    sp = min(K, S - j0)
                    ax = ppool.tile([128, K, HC], BF16, tag="ax")
                    nc.gpsimd.tensor_mul(
                        out=ax[:, :sp, :].rearrange("p j (h c) -> p j h c", h=H),
                        in0=yblk[:, j0:j0 + sp, :].rearrange("p j (h c) -> p j h c", h=H),
                        in1=alpha[:, j0 * H:(j0 + sp) * H]
                            .rearrange("p (j h) -> p j h", j=sp)
                            .unsqueeze(3).broadcast_to([128, sp, H, C]))
                    # pair-sum slots on vector to halve the PE matmul count
                    npair = sp // 2
                    pr = ppool.tile([128, K // 2 + 1, HC], BF16, tag="pr")
                    if npair:
                        nc.vector.tensor_add(
                            out=pr[:, :npair, :],
                            in0=ax[:, 0:2 * npair:2, :],
                            in1=ax[:, 1:2 * npair:2, :])
                    nsing = sp - 2 * npair
                    for jj in range(npair):
                        nmm += 1
                        nc.tensor.matmul(out=hps[:], lhsT=identbf[:],
                                         rhs=pr[:, jj, :],
                                         start=nmm == 1, stop=nmm == mm_total,
                                         skip_group_check=True)
                    if nsing:
                        nmm += 1
                        nc.tensor.matmul(out=hps[:], lhsT=identbf[:],
                                         rhs=ax[:, sp - 1, :],
                                         start=nmm == 1, stop=nmm == mm_total,
                                         skip_group_check=True)

                # ---- corrections: h = hps - xr - (sum_j alpha*ea)*We + bias
                # (xr - b1 folded into t_xrb host-side)
                eaa = bpool.tile([128, SMAX * H], F32, tag="eaa")
                nc.vector.tensor_mul(
                    out=eaa[:, :S * H].rearrange("p (j h) -> p j h", j=S),
                    in0=alpha[:, :S * H].rearrange("p (j h) -> p j h", j=S),
                    in1=eab[:, s0:s0 + S].unsqueeze(2).broadcast_to([128, S, H]))
                eam = bpool.tile([128, H], F32, tag="eam")
                nc.vector.tensor_reduce(
                    out=eam[:],
                    in_=eaa[:, :S * H].rearrange("p (j h) -> p h j", j=S),
                    axis=mybir.AxisListType.X, op=mybir.AluOpType.add)
                eamx = bpool.tile([128, HC], F32, tag="eamx")
                nc.vector.tensor_copy(
                    out=eamx[:].rearrange("p (h c) -> p h c", h=H),
                    in_=eam[:].unsqueeze(2).broadcast_to([128, H, C]))
                corr = bpool.tile([128, HC], F32, tag="corr")
                nc.vector.scalar_tensor_tensor(
                    out=corr[:], in0=webf[:], scalar=1.0, in1=eamx[:],
                    op0=mybir.AluOpType.mult, op1=mybir.AluOpType.mult)
                xrv = xrbb[:, b * HC:(b + 1) * HC]
                nc.vector.tensor_add(out=corr[:], in0=corr[:], in1=xrv)
                hblk = bpool.tile([128, HC], F32, tag="hblk")
                nc.vector.tensor_sub(out=hblk[:], in0=hps[:], in1=corr[:])
                # ---- true ELU on scalar: relu(x) + exp(-relu(-x)) - 1
                tneg = bpool.tile([128, HC], F32, tag="tneg")
                nc.scalar.activation(out=tneg[:], in_=hblk[:],
                                     func=mybir.ActivationFunctionType.Relu,
                                     scale=-1.0)
                nc.scalar.activation(out=tneg[:], in_=tneg[:],
                                     func=mybir.ActivationFunctionType.Exp,
                                     scale=-1.0)
                nc.scalar.activation(out=hblk[:], in_=hblk[:],
                                     func=mybir.ActivationFunctionType.Relu)
                nc.vector.tensor_add(out=hblk[:], in0=hblk[:], in1=tneg[:])
                hbf = bpool.tile([128, HC], BF16, tag="hbf")
                nc.vector.tensor_scalar_add(out=hbf[:], in0=hblk[:], scalar1=-1.0)

                # ---- per-block tail
                tp = psum.tile([128, 128], BF16, tag="tp")
                nc.tensor.transpose(out=tp[:], in_=hbf[:], identity=identbf[:])
                hT = bpool.tile([128, 128], BF16, tag="hT")
                nc.scalar.copy(out=hT[:], in_=tp[:])
                if layer == 1:
                    mm2 = psum.tile([128, HC], F32, tag="mm2")
                    nc.tensor.matmul(out=mm2[:], lhsT=hT[:], rhs=wl2[:],
                                     start=True, stop=True)
                    xl2sb = bpool.tile([128, HC], BF16, tag="xl2sb")
                    nc.vector.tensor_add(out=xl2sb[:], in0=mm2[:], in1=bl2row[:])
                    nc.sync.dma_start(out=o_xl2.ap()[b * 128:(b + 1) * 128, :],
                                      in_=xl2sb[:])
                    mm3 = psum.tile([128, HC], F32, tag="mm3")
                    nc.tensor.matmul(out=mm3[:], lhsT=hT[:], rhs=wr2[:],
                                     start=True, stop=True)
                    xr2sb = bpool.tile([128, HC], BF16, tag="xr2sb")
                    nc.vector.tensor_add(out=xr2sb[:], in0=mm3[:], in1=br2row[:])
                    nc.sync.dma_start(out=o_xr2.ap()[b], in_=xr2sb[:])
                else:
                    mp1 = psum.tile([32, 128], F32, tag="mp1")
                    nc.tensor.matmul(out=mp1[:], lhsT=w1[:], rhs=hT[:],
                                     start=True, stop=True)
                    r1 = bpool.tile([32, 128], BF16, tag="r1")
                    nc.scalar.activation(out=r1[:], in_=mp1[:],
                                         func=mybir.ActivationFunctionType.Relu,
                                         bias=c1[:, 0:1])
                    mp2 = psum.tile([32, 128], F32, tag="mp2")
                    nc.tensor.matmul(out=mp2[:], lhsT=w2[:], rhs=r1[:],
                                     start=True, stop=True)
                    r2 = bpool.tile([32, 128], BF16, tag="r2")
                    nc.scalar.activation(out=r2[:], in_=mp2[:],
                                         func=mybir.ActivationFunctionType.Relu,
                                         bias=c2[:, 0:1])
                    mp3 = psum.tile([OUT, 128], F32, tag="mp3")
                    nc.tensor.matmul(out=mp3[:], lhsT=w3[:], rhs=r2[:],
                                     start=True, stop=True)
                    r3 = bpool.tile([OUT, 128], F32, tag="r3")
                    nc.vector.tensor_scalar_add(out=r3[:], in0=mp3[:],
                                                scalar1=c3[:, 0:1])
                    nc.sync.dma_start(out=o_out.ap()[b], in_=r3[:])
    nc.compile()
    return nc


# ---------------------------------------------------------------- kernel

def kernel(x, edge_index, edge_attr,
           Wl1, bl1, Wr1, br1, We1, att1, b1,
           Wl2, bl2, Wr2, br2, We2, att2, b2,
           W1, c1, W2, c2, W3, c3):
    x = np.asarray(x, np.float32)
    edge_index = np.asarray(edge_index, np.int32)
    edge_attr = np.asarray(edge_attr, np.float32)
    f = lambda a: np.asarray(a, np.float32)
    Wl1, bl1, Wr1, br1, We1 = f(Wl1), f(bl1), f(Wr1), f(br1), f(We1)
    att1, b1 = f(att1), f(b1)
    Wl2, bl2, Wr2, br2, We2 = f(Wl2), f(bl2), f(Wr2), f(br2), f(We2)
    att2, b2 = f(att2), f(b2)
    W1, c1, W2, c2, W3, c3 = f(W1), f(c1), f(W2), f(c2), f(W3), f(c3)

    cores, shared = _build_schedule(edge_index, edge_attr)

    # host-side layer-1 tables (f32; ytab blob casts to bf16 at the end)
    xl1 = x @ Wl1.T + bl1                      # [N, 128]
    xr1 = x @ Wr1.T + br1
    we1f = We1[:, 0]
    we2f = We2[:, 0]

    row = lambda v, dt: np.tile(v[None, :], (128, 1)).astype(dt)
    web1 = row(we1f, np.float32)
    attb1 = row(att1.reshape(-1), BF)
    web2 = row(we2f, np.float32)
    attb2 = row(att2.reshape(-1), BF)
    bl2row = row(bl2, np.float32)
    br2row = row(br2, np.float32)

    def xrb_blob(xr, bias):
        """[128, NBLK*HC] f32: row-permuted (xr - bias) tiles (h subtracts
        this, so folding -bias adds the layer bias), column-blocked."""
        out = np.zeros((NCORES, 128, NBLK * HC), np.float32)
        for k in range(NCORES):
            perm_pad = cores[k]["perm_pad"]
            safe = np.clip(perm_pad, 0, N - 1)
            v = np.asarray(xr, np.float32)[safe] - bias[None, :]
            v[perm_pad < 0] = 0
            out[k] = v.reshape(NBLK, 128, HC).transpose(1, 0, 2).reshape(128, -1)
        return out

    shared["_layer"] = 1
    ncA = _build_launch(1, shared)
    xrb1 = xrb_blob(xr1, b1)
    in_maps = []
    for k in range(NCORES):
        in_maps.append({
            "t_ytab": _ytab_blob(cores[k], xl1, xr1, we1f, att1, shared),
            "t_ea": cores[k]["eablob"],
            "t_xrb": xrb1[k],
            "t_webf": web1, "t_attb": attb1,
            "t_wl2": Wl2.T.astype(BF).copy(), "t_wr2": Wr2.T.astype(BF).copy(),
            "t_bl2row": bl2row, "t_br2row": br2row,
        })
    resA = _run(ncA, in_maps)

    # exchange: assemble natural-order layer-2 tables
    xl2 = np.zeros((N, HC), np.float32)
    xr2 = np.zeros((N, HC), np.float32)
    for k in range(NCORES):
        perm_pad = cores[k]["perm_pad"]
        valid = perm_pad >= 0
        xl2[perm_pad[valid]] = resA[k]["o_xl2"][valid].astype(np.float32)
        xr2[perm_pad[valid]] = resA[k]["o_xr2"].reshape(NPAD, HC)[valid].astype(np.float32)

    shared["_layer"] = 2
    ncB = _build_launch(2, shared)
    xrb2 = xrb_blob(xr2, b2)
    in_mapsB = []
    for k in range(NCORES):
        in_mapsB.append({
            "t_ytab": _ytab_blob(cores[k], xl2, xr2, we2f, att2, shared),
            "t_ea": cores[k]["eablob"],
            "t_xrb": xrb2[k],
            "t_webf": web2, "t_attb": attb2,
            "t_w1": W1.T.astype(BF).copy(), "t_w2": W2.T.astype(BF).copy(),
            "t_w3": W3.T.astype(BF).copy(),
            "t_c1": c1.reshape(32, 1), "t_c2": c2.reshape(32, 1),
            "t_c3": c3.reshape(OUT, 1),
        })
    resB = _run(ncB, in_mapsB)

    out = np.zeros((N, OUT), np.float32)
    for k in range(NCORES):
        perm_pad = cores[k]["perm_pad"]
        valid = perm_pad >= 0
        o = resB[k]["o_out"].transpose(0, 2, 1).reshape(NPAD, OUT)
        out[perm_pad[valid]] = o[valid]
    return out


# revision 30
# speedup vs baseline: 1.2586x; 1.2586x over previous
"""GATv2 localization model on 8 Trainium2 NeuronCores (Bass/Tile), v3.

Strategy (dst-sharded message passing, bf16 edge pipeline):
  - Nodes sharded across 8 cores by dst (6250 each); per core, nodes are
    degree-sorted into 49 blocks of 128 and incoming edges are packed into
    per-block slot columns (slot q of block b holds edge q of each node).
  - The attention input y[e] = xl[src] + xr[dst] + We*ea is assembled on
    the host in slot order and streamed sequentially (one direct DMA per
    block) — on TRN2, software-DGE indirect gathers cost ~6ns/row on the
    gpsimd sequencer, so a prearranged affine stream is the only way to
    run this at memory bandwidth.
  - Per pass (8 slots): scalar applies leaky-relu, vector applies att and
    reduces to logits. Softmax (exp/mask/denominator) runs once per
    block; alpha is pre-normalized so no divide is needed downstream.
  - Aggregation uses sum_j alpha_j*xl_j = sum_j alpha_j*y_j - xr -
    (sum_j alpha_j ea_j)*We: gpsimd forms alpha*y, vector pair-sums
    slots, and the tensor engine accumulates into PSUM via identity-lhsT
    matmuls; the correction terms are per-block vector ops.
  - Layer-1 tables are host-computed; layer-2 tables are built on-device
    from h1 (bf16, true ELU to keep bf16 precision) and exchanged through
    the host between the two launches. MLP head on-device per block.
"""

import os
import numpy as np
import ml_dtypes

import concourse.bacc as bacc
import concourse.tile as tile
import concourse.mybir as mybir
from concourse import bass
from concourse.bass_utils import run_bass_kernel_spmd
from concourse.masks import make_identity

F32 = mybir.dt.float32
BF16 = mybir.dt.bfloat16
BF = ml_dtypes.bfloat16

N = 50000
E = 800000
IN = 16
H1 = 4
C1 = 32
HC = 128
OUT = 2
NCORES = 8
NSHARD = N // NCORES          # 6250
NBLK = (NSHARD + 127) // 128  # 49
NPAD = NBLK * 128             # 6272
K = 8                         # slots per pass

_EXEC_NS = []                 # per-launch HW exec time when GAT_TRACE=1


def _maybe_install_trace_hook():
    if os.environ.get("GAT_TRACE", "0") != "1":
        return False
    import contextlib, ctypes, sys, types
    if "antenv.axon_hooks" not in sys.modules:
        def _mk(so_path):
            lib = ctypes.CDLL(so_path)
            if not hasattr(lib, "axon_start_nrt_profile"):
                return None
            lib.axon_start_nrt_profile.argtypes = [ctypes.POINTER(ctypes.c_int64), ctypes.c_size_t]
            lib.axon_start_nrt_profile.restype = ctypes.c_int64
            lib.axon_stop_nrt_profile.argtypes = [ctypes.c_char_p]
            lib.axon_stop_nrt_profile.restype = ctypes.c_int64

            @contextlib.contextmanager
            def _hook(output_dir, device_ids):
                import jax
                jax.devices()
                if device_ids:
                    ids = (ctypes.c_int64 * len(device_ids))(*device_ids)
                    rc = lib.axon_start_nrt_profile(ids, len(device_ids))
                else:
                    rc = lib.axon_start_nrt_profile(None, 0)
                if rc != 0:
                    raise RuntimeError(f"axon_start_nrt_profile rc={rc}")
                try:
                    yield
                finally:
                    n = lib.axon_stop_nrt_profile(str(output_dir).encode())
                    if n < 0:
                        raise RuntimeError(f"axon_stop_nrt_profile rc={n}")
            return _hook

        hook = _mk("/opt/axon/libaxon_pjrt.so")
        mod = types.ModuleType("antenv.axon_hooks")
        mod.get_axon_ntff_profile_hook = lambda: hook
        mod.set_axon_ntff_profile_hook = lambda h: None
        sys.modules["antenv.axon_hooks"] = mod
        import concourse.bass_utils as bu
        bu.upload_artifacts = lambda tmpdir: tmpdir
    return True


def _run(nc, in_maps):
    trace = _maybe_install_trace_hook()
    if trace:
        import tempfile
        res = run_bass_kernel_spmd(nc, in_maps, core_ids=list(range(NCORES)),
                                   trace=True, tmpdir=tempfile.mkdtemp())
        _EXEC_NS.append(res.exec_time_ns)
    else:
        res = run_bass_kernel_spmd(nc, in_maps, core_ids=list(range(NCORES)))
    return res.results


# ---------------------------------------------------------------- schedule

def _build_schedule(edge_index, edge_attr):
    """Degree-sorted blocks, slot layout, per-core slot-ordered edge refs."""
    src = edge_index[0].astype(np.int64)
    dst = edge_index[1].astype(np.int64)
    ea = edge_attr[:, 0].astype(np.float32)

    deg = np.bincount(dst, minlength=N)
    cores = []
    for k in range(NCORES):
        lo, hi = k * NSHARD, (k + 1) * NSHARD
        nodes = np.arange(lo, hi)
        order = np.argsort(-deg[lo:hi], kind="stable")
        perm = nodes[order]                       # block row -> global node id
        perm_pad = np.concatenate([perm, np.full(NPAD - NSHARD, -1, np.int64)])
        cores.append({"perm_pad": perm_pad})

    # shared slot counts per block (max over cores)
    SLOTS = np.zeros(NBLK, np.int64)
    for k in range(NCORES):
        perm_pad = cores[k]["perm_pad"]
        d = np.where(perm_pad >= 0, deg[np.clip(perm_pad, 0, N - 1)], 0)
        SLOTS = np.maximum(SLOTS, d.reshape(NBLK, 128).max(axis=1))
    SLOTS = np.maximum(SLOTS, 1)
    PB = (SLOTS + K - 1) // K
    scol0 = np.zeros(NBLK + 1, np.int64)
    scol0[1:] = np.cumsum(SLOTS)
    TOTS = int(SLOTS.sum())

    # edge lists grouped by dst
    e_order = np.argsort(dst, kind="stable")
    src_s, ea_s = src[e_order], ea[e_order]
    starts = np.searchsorted(dst[e_order], np.arange(N + 1))

    # per-core slot-ordered edge references: for slot column (b, q), row r:
    # flat position p = (scol0[b]+q)*128 + r -> source node id (or -1 pad)
    for k in range(NCORES):
        perm_pad = cores[k]["perm_pad"]
        srcmap = np.full(TOTS * 128, -1, np.int64)   # [(scol, r)]
        eamap = np.zeros((128, TOTS), np.float32)    # [r, scol]
        mkmap = np.zeros((128, TOTS), np.float32)
        for b in range(NBLK):
            rows = perm_pad[b * 128:(b + 1) * 128]
            for r in range(128):
                n = rows[r]
                if n < 0:
                    continue
                s0e, s1e = starts[n], starts[n + 1]
                d = s1e - s0e
                if d == 0:
                    continue
                cols = scol0[b] + np.arange(d)
                srcmap[cols * 128 + r] = src_s[s0e:s1e]
                eamap[r, cols] = ea_s[s0e:s1e]
                mkmap[r, cols] = 1.0
        cores[k]["srcmap"] = srcmap
        cores[k]["eamap"] = eamap
        cores[k]["mkblob"] = mkmap
        cores[k]["eablob"] = eamap.astype(BF)

    shared = {"SLOTS": SLOTS, "PB": PB, "scol0": scol0, "TOTS": TOTS}
    return cores, shared


def _ytab_blob(core, xl, xr, we, att_flat, shared):
    """Slot-ordered [128, TOTS*HC] bf16 streams: (ytab, yagg).

    ytab = xl[src]+xr[dst]+We*ea feeds the attention logits; its pad
    slots are filled with -t*sign(att) per channel, which makes the
    device-computed logit sum(att*lrelu(y)) <= -40 for every head, so
    exp() underflows to ~0 and no mask multiply is needed.
    yagg = xl[src]+xr[dst] feeds the aggregation (sum_j alpha_j*yagg_j =
    sum_j alpha_j*xl[src_j] + xr[dst]); its pad slots are zero."""
    TOTS = shared["TOTS"]
    srcmap = core["srcmap"]                  # [(scol)*128 + r]
    perm_pad = core["perm_pad"]
    safe_src = np.clip(srcmap, 0, N - 1)
    yt = xl[safe_src].astype(np.float32)     # [(scol, r), HC]
    yt = yt.reshape(TOTS, 128, HC)
    # xr[dst]: dst of (b, q, r) is block row r of block b
    scol0 = shared["scol0"]
    safe = np.clip(perm_pad, 0, N - 1)
    v = xr[safe].astype(np.float32)
    v[perm_pad < 0] = 0.0
    xrv = v.reshape(NBLK, 128, HC)
    for b in range(NBLK):
        yt[scol0[b]:scol0[b + 1]] += xrv[b][None, :, :]
    pad = srcmap.reshape(TOTS, 128) < 0
    yt[pad] = 0.0
    yagg = np.ascontiguousarray(
        yt.transpose(1, 0, 2).reshape(128, TOTS * HC)).astype(BF)
    yt += core["eamap"].T[:, :, None] * we[None, None, :]
    # pad-slot poison logits: coefficient per head of -t is
    # sum_c(0.2*max(att,0) - min(att,0)); choose t so every head's pad
    # logit is <= -40.
    a = att_flat.reshape(-1).astype(np.float32)
    heads = H1 if shared["_layer"] == 1 else 1
    ah = a.reshape(heads, -1)
    coef = (0.2 * np.maximum(ah, 0) - np.minimum(ah, 0)).sum(axis=1)
    t = min(40.0 / max(coef.min(), 1e-3), 3e4)
    padvec = (-t * np.sign(a)).astype(np.float32)
    yt[pad] = padvec
    ytab = np.ascontiguousarray(
        yt.transpose(1, 0, 2).reshape(128, TOTS * HC)).astype(BF)
    return ytab, yagg


# ---------------------------------------------------------------- launches

def _build_launch(layer, shared):
    nc = bacc.Bacc("TRN2", target_bir_lowering=False, debug=False,
                   num_devices=NCORES)
    H = H1 if layer == 1 else 1
    C = HC // H
    SLOTS, PB, scol0 = shared["SLOTS"], shared["PB"], shared["scol0"]
    TOTS = shared["TOTS"]
    SMAX = int(SLOTS.max())

    t_ytab = nc.dram_tensor("t_ytab", [128, TOTS * HC], BF16, kind="ExternalInput")
    t_yagg = nc.dram_tensor("t_yagg", [128, TOTS * HC], BF16, kind="ExternalInput")
    t_xrb = nc.dram_tensor("t_xrb", [128, NBLK * HC], F32, kind="ExternalInput")
    t_attb = nc.dram_tensor("t_attb", [128, HC], BF16, kind="ExternalInput")
    if layer == 1:
        t_wl2 = nc.dram_tensor("t_wl2", [HC, HC], BF16, kind="ExternalInput")
        t_wr2 = nc.dram_tensor("t_wr2", [HC, HC], BF16, kind="ExternalInput")
        t_bl2row = nc.dram_tensor("t_bl2row", [128, HC], F32, kind="ExternalInput")
        t_br2row = nc.dram_tensor("t_br2row", [128, HC], F32, kind="ExternalInput")
        o_xl2 = nc.dram_tensor("o_xl2", [NPAD, HC], BF16, kind="ExternalOutput")
        o_xr2 = nc.dram_tensor("o_xr2", [NBLK, 128, HC], BF16, kind="ExternalOutput")
    else:
        t_w1 = nc.dram_tensor("t_w1", [HC, 32], BF16, kind="ExternalInput")
        t_w2 = nc.dram_tensor("t_w2", [32, 32], BF16, kind="ExternalInput")
        t_w3 = nc.dram_tensor("t_w3", [32, OUT], BF16, kind="ExternalInput")
        t_c1 = nc.dram_tensor("t_c1", [32, 1], F32, kind="ExternalInput")
        t_c2 = nc.dram_tensor("t_c2", [32, 1], F32, kind="ExternalInput")
        t_c3 = nc.dram_tensor("t_c3", [OUT, 1], F32, kind="ExternalInput")
        o_out = nc.dram_tensor("o_out", [NBLK, OUT, 128], F32, kind="ExternalOutput")

    with tile.TileContext(nc) as tc:
        with tc.tile_pool(name="const", bufs=1) as cpool, \
             tc.tile_pool(name="blk", bufs=2) as bpool, \
             tc.tile_pool(name="pas", bufs=3) as ppool, \
             tc.tile_pool(name="psum", bufs=1, space="PSUM") as psum:
            attb = cpool.tile([128, HC], BF16)
            nc.sync.dma_start(out=attb[:], in_=t_attb.ap())
            xrbb = cpool.tile([128, NBLK * HC], F32)
            nc.sync.dma_start(out=xrbb[:], in_=t_xrb.ap())
            identbf = cpool.tile([128, 128], BF16)
            make_identity(nc, identbf[:])
            if layer == 1:
                wl2 = cpool.tile([HC, HC], BF16)
                nc.sync.dma_start(out=wl2[:], in_=t_wl2.ap())
                wr2 = cpool.tile([HC, HC], BF16)
                nc.sync.dma_start(out=wr2[:], in_=t_wr2.ap())
                bl2row = cpool.tile([128, HC], F32)
                nc.sync.dma_start(out=bl2row[:], in_=t_bl2row.ap())
                br2row = cpool.tile([128, HC], F32)
                nc.sync.dma_start(out=br2row[:], in_=t_br2row.ap())
            else:
                w1 = cpool.tile([HC, 32], BF16)
                nc.sync.dma_start(out=w1[:], in_=t_w1.ap())
                w2 = cpool.tile([32, 32], BF16)
                nc.sync.dma_start(out=w2[:], in_=t_w2.ap())
                w3 = cpool.tile([32, OUT], BF16)
                nc.sync.dma_start(out=w3[:], in_=t_w3.ap())
                c1 = cpool.tile([32, 1], F32)
                nc.sync.dma_start(out=c1[:], in_=t_c1.ap())
                c2 = cpool.tile([32, 1], F32)
                nc.sync.dma_start(out=c2[:], in_=t_c2.ap())
                c3 = cpool.tile([OUT, 1], F32)
                nc.sync.dma_start(out=c3[:], in_=t_c3.ap())

            for b in range(NBLK):
                S = int(SLOTS[b])
                s0 = int(scol0[b])
                yblk = bpool.tile([128, SMAX, HC], BF16, tag="yblk")
                nc.sync.dma_start(
                    out=yblk[:, :S, :],
                    in_=t_ytab.ap()[:, s0 * HC:(s0 + S) * HC]
                        .rearrange("p (s c) -> p s c", c=HC))
                gblk = bpool.tile([128, SMAX, HC], BF16, tag="gblk")
                nc.scalar.dma_start(
                    out=gblk[:, :S, :],
                    in_=t_yagg.ap()[:, s0 * HC:(s0 + S) * HC]
                        .rearrange("p (s c) -> p s c", c=HC))
                wall = bpool.tile([128, SMAX * H], F32, tag="wall")

                for pl in range(int(PB[b])):
                    j0 = pl * K
                    sp = min(K, S - j0)
                    m = ppool.tile([128, K, HC], BF16, tag="m")
                    nc.scalar.activation(out=m[:, :sp, :], in_=yblk[:, j0:j0 + sp, :],
                                         func=mybir.ActivationFunctionType.Prelu,
                                         alpha=0.2)
                    mm = ppool.tile([128, K, HC], BF16, tag="mm")
                    nc.vector.tensor_mul(
                        out=mm[:, :sp, :], in0=m[:, :sp, :],
                        in1=attb[:].unsqueeze(1).broadcast_to([128, sp, HC]))
                    nc.vector.tensor_reduce(
                        out=wall[:, j0 * H:(j0 + sp) * H],
                        in_=mm[:, :sp, :].rearrange("p j (h c) -> p j h c", h=H),
                        axis=mybir.AxisListType.X, op=mybir.AluOpType.add)

                # ---- block-level softmax weights (pre-normalized alpha)
                # (pad slots carry logit ~ -46 from the host ytab, so no mask
                #  multiply and no epsilon are needed)
                nc.scalar.activation(out=wall[:, :S * H], in_=wall[:, :S * H],
                                     func=mybir.ActivationFunctionType.Exp)
                dacc = bpool.tile([128, H], F32, tag="dacc")
                nc.vector.tensor_reduce(
                    out=dacc[:],
                    in_=wall[:, :S * H].rearrange("p (j h) -> p h j", j=S),
                    axis=mybir.AxisListType.X, op=mybir.AluOpType.add)
                nc.vector.reciprocal(out=dacc[:], in_=dacc[:])
                alpha = bpool.tile([128, SMAX * H], BF16, tag="alpha")
                nc.vector.tensor_mul(
                    out=alpha[:, :S * H].rearrange("p (j h) -> p j h", j=S),
                    in0=wall[:, :S * H].rearrange("p (j h) -> p j h", j=S),
                    in1=dacc[:].unsqueeze(1).broadcast_to([128, S, H]))

                # ---- phase 2: Sum_j alpha_j*y_j via gpsimd mul + pair-sum + PE
                hps = psum.tile([128, HC], F32, tag="hps")
                nmm = 0
                mm_total = sum((min(K, S - pl * K) + 1) // 2
                               for pl in range(int(PB[b])))
                for pl in range(int(PB[b])):
                    j0 = pl * K
                    sp = min(K, S - j0)
                    ax = ppool.tile([128, K, HC], BF16, tag="ax")
                    nc.gpsimd.tensor_mul(
                        out=ax[:, :sp, :].rearrange("p j (h c) -> p j h c", h=H),
                        in0=gblk[:, j0:j0 + sp, :].rearrange("p j (h c) -> p j h c", h=H),
                        in1=alpha[:, j0 * H:(j0 + sp) * H]
                            .rearrange("p (j h) -> p j h", j=sp)
                            .unsqueeze(3).broadcast_to([128, sp, H, C]))
                    # pair-sum slots to halve the PE matmul count
                    npair = sp // 2
                    pr = ppool.tile([128, K // 2 + 1, HC], BF16, tag="pr")
                    if npair:
                        peng = nc.vector if pl % 2 == 0 else nc.gpsimd
                        peng.tensor_add(
                            out=pr[:, :npair, :],
                            in0=ax[:, 0:2 * npair:2, :],
                            in1=ax[:, 1:2 * npair:2, :])
                    nsing = sp - 2 * npair
                    for jj in range(npair):
                        nmm += 1
                        nc.tensor.matmul(out=hps[:], lhsT=identbf[:],
                                         rhs=pr[:, jj, :],
                                         start=nmm == 1, stop=nmm == mm_total,
                                         skip_group_check=True)
                    if nsing:
                        nmm += 1
                        nc.tensor.matmul(out=hps[:], lhsT=identbf[:],
                                         rhs=ax[:, sp - 1, :],
                                         start=nmm == 1, stop=nmm == mm_total,
                                         skip_group_check=True)

                # ---- correction: yagg = xl[src]+xr[dst], sum_h alpha = 1, so
                # h = hps - (xr - bias)   (xr - bias folded into t_xrb)
                xrv = xrbb[:, b * HC:(b + 1) * HC]
                hblk = bpool.tile([128, HC], F32, tag="hblk")
                nc.vector.tensor_sub(out=hblk[:], in0=hps[:], in1=xrv)
                # ---- true ELU on scalar: relu(x) + exp(-relu(-x)) - 1
                tneg = bpool.tile([128, HC], F32, tag="tneg")
                nc.scalar.activation(out=tneg[:], in_=hblk[:],
                                     func=mybir.ActivationFunctionType.Relu,
                                     scale=-1.0)
                nc.scalar.activation(out=tneg[:], in_=tneg[:],
                                     func=mybir.ActivationFunctionType.Exp,
                                     scale=-1.0)
                nc.scalar.activation(out=hblk[:], in_=hblk[:],
                                     func=mybir.ActivationFunctionType.Relu)
                nc.vector.tensor_add(out=hblk[:], in0=hblk[:], in1=tneg[:])
                hbf = bpool.tile([128, HC], BF16, tag="hbf")
                nc.vector.tensor_scalar_add(out=hbf[:], in0=hblk[:], scalar1=-1.0)

                # ---- per-block tail
                tp = psum.tile([128, 128], BF16, tag="tp")
                nc.tensor.transpose(out=tp[:], in_=hbf[:], identity=identbf[:])
                hT = bpool.tile([128, 128], BF16, tag="hT")
                nc.scalar.copy(out=hT[:], in_=tp[:])
                if layer == 1:
                    mm2 = psum.tile([128, HC], F32, tag="mm2")
                    nc.tensor.matmul(out=mm2[:], lhsT=hT[:], rhs=wl2[:],
                                     start=True, stop=True)
                    xl2sb = bpool.tile([128, HC], BF16, tag="xl2sb")
                    nc.vector.tensor_add(out=xl2sb[:], in0=mm2[:], in1=bl2row[:])
                    nc.sync.dma_start(out=o_xl2.ap()[b * 128:(b + 1) * 128, :],
                                      in_=xl2sb[:])
                    mm3 = psum.tile([128, HC], F32, tag="mm3")
                    nc.tensor.matmul(out=mm3[:], lhsT=hT[:], rhs=wr2[:],
                                     start=True, stop=True)
                    xr2sb = bpool.tile([128, HC], BF16, tag="xr2sb")
                    nc.vector.tensor_add(out=xr2sb[:], in0=mm3[:], in1=br2row[:])
                    nc.sync.dma_start(out=o_xr2.ap()[b], in_=xr2sb[:])
                else:
                    mp1 = psum.tile([32, 128], F32, tag="mp1")
                    nc.tensor.matmul(out=mp1[:], lhsT=w1[:], rhs=hT[:],
                                     start=True, stop=True)
                    r1 = bpool.tile([32, 128], BF16, tag="r1")
                    nc.scalar.activation(out=r1[:], in_=mp1[:],
                                         func=mybir.ActivationFunctionType.Relu,
                                         bias=c1[:, 0:1])
                    mp2 = psum.tile([32, 128], F32, tag="mp2")
                    nc.tensor.matmul(out=mp2[:], lhsT=w2[:], rhs=r1[:],
                                     start=True, stop=True)
                    r2 = bpool.tile([32, 128], BF16, tag="r2")
                    nc.scalar.activation(out=r2[:], in_=mp2[:],
                                         func=mybir.ActivationFunctionType.Relu,
                                         bias=c2[:, 0:1])
                    mp3 = psum.tile([OUT, 128], F32, tag="mp3")
                    nc.tensor.matmul(out=mp3[:], lhsT=w3[:], rhs=r2[:],
                                     start=True, stop=True)
                    r3 = bpool.tile([OUT, 128], F32, tag="r3")
                    nc.vector.tensor_scalar_add(out=r3[:], in0=mp3[:],
                                                scalar1=c3[:, 0:1])
                    nc.sync.dma_start(out=o_out.ap()[b], in_=r3[:])
    nc.compile()
    return nc


# ---------------------------------------------------------------- kernel

def kernel(x, edge_index, edge_attr,
           Wl1, bl1, Wr1, br1, We1, att1, b1,
           Wl2, bl2, Wr2, br2, We2, att2, b2,
           W1, c1, W2, c2, W3, c3):
    x = np.asarray(x, np.float32)
    edge_index = np.asarray(edge_index, np.int32)
    edge_attr = np.asarray(edge_attr, np.float32)
    f = lambda a: np.asarray(a, np.float32)
    Wl1, bl1, Wr1, br1, We1 = f(Wl1), f(bl1), f(Wr1), f(br1), f(We1)
    att1, b1 = f(att1), f(b1)
    Wl2, bl2, Wr2, br2, We2 = f(Wl2), f(bl2), f(Wr2), f(br2), f(We2)
    att2, b2 = f(att2), f(b2)
    W1, c1, W2, c2, W3, c3 = f(W1), f(c1), f(W2), f(c2), f(W3), f(c3)

    cores, shared = _build_schedule(edge_index, edge_attr)

    # host-side layer-1 tables (f32; ytab blob casts to bf16 at the end)
    xl1 = x @ Wl1.T + bl1                      # [N, 128]
    xr1 = x @ Wr1.T + br1
    we1f = We1[:, 0]
    we2f = We2[:, 0]

    row = lambda v, dt: np.tile(v[None, :], (128, 1)).astype(dt)
    web1 = row(we1f, np.float32)
    attb1 = row(att1.reshape(-1), BF)
    web2 = row(we2f, np.float32)
    attb2 = row(att2.reshape(-1), BF)
    bl2row = row(bl2, np.float32)
    br2row = row(br2, np.float32)

    def xrb_blob(xr, bias):
        """[128, NBLK*HC] f32: row-permuted (xr - bias) tiles (h subtracts
        this, so folding -bias adds the layer bias), column-blocked."""
        out = np.zeros((NCORES, 128, NBLK * HC), np.float32)
        for k in range(NCORES):
            perm_pad = cores[k]["perm_pad"]
            safe = np.clip(perm_pad, 0, N - 1)
            v = np.asarray(xr, np.float32)[safe] - bias[None, :]
            v[perm_pad < 0] = 0
            out[k] = v.reshape(NBLK, 128, HC).transpose(1, 0, 2).reshape(128, -1)
        return out

    shared["_layer"] = 1
    ncA = _build_launch(1, shared)
    xrb1 = xrb_blob(xr1, b1)
    in_maps = []
    for k in range(NCORES):
        ytab, yagg = _ytab_blob(cores[k], xl1, xr1, we1f, att1, shared)
        in_maps.append({
            "t_ytab": ytab, "t_yagg": yagg,
            "t_xrb": xrb1[k],
            "t_attb": attb1,
            "t_wl2": Wl2.T.astype(BF).copy(), "t_wr2": Wr2.T.astype(BF).copy(),
            "t_bl2row": bl2row, "t_br2row": br2row,
        })
    resA = _run(ncA, in_maps)

    # exchange: assemble natural-order layer-2 tables
    xl2 = np.zeros((N, HC), np.float32)
    xr2 = np.zeros((N, HC), np.float32)
    for k in range(NCORES):
        perm_pad = cores[k]["perm_pad"]
        valid = perm_pad >= 0
        xl2[perm_pad[valid]] = resA[k]["o_xl2"][valid].astype(np.float32)
        xr2[perm_pad[valid]] = resA[k]["o_xr2"].reshape(NPAD, HC)[valid].astype(np.float32)

    shared["_layer"] = 2
    ncB = _build_launch(2, shared)
    xrb2 = xrb_blob(xr2, b2)
    in_mapsB = []
    for k in range(NCORES):
        ytabB, yaggB = _ytab_blob(cores[k], xl2, xr2, we2f, att2, shared)
        in_mapsB.append({
            "t_ytab": ytabB, "t_yagg": yaggB,
            "t_xrb": xrb2[k],
            "t_attb": attb2,
            "t_w1": W1.T.astype(BF).copy(), "t_w2": W2.T.astype(BF).copy(),
            "t_w3": W3.T.astype(BF).copy(),
            "t_c1": c1.reshape(32, 1), "t_c2": c2.reshape(32, 1),
            "t_c3": c3.reshape(OUT, 1),
        })
    resB = _run(ncB, in_mapsB)

    out = np.zeros((N, OUT), np.float32)
    for k in range(NCORES):
        perm_pad = cores[k]["perm_pad"]
        valid = perm_pad >= 0
        o = resB[k]["o_out"].transpose(0, 2, 1).reshape(NPAD, OUT)
        out[perm_pad[valid]] = o[valid]
    return out
